# revision 3
# baseline (speedup 1.0000x reference)
"""MoE ACT block (nn_MoEACTBlock_62225486185201) on 8 Trainium2 NeuronCores.

Strategy: expert-parallel with top-2 routing sparsity.
  - Host: router matmul + top-2 + renormalize (0.01% of FLOPs), builds
    per-expert token index lists padded to a fixed capacity.
  - Device (core e = expert e): indirect-DMA gather of its tokens, LayerNorm,
    PE transpose to feature-major, h = gelu(xn @ w1 + b1), y = w_tok * (h @ w2),
    PE transpose back to token-major, write y rows.
  - Host: combine out = x + sum_e scatter(y_e) (+ b2 term), since each token is
    processed by exactly 2 experts living on different cores.

LayerNorm gamma/beta are folded into w1/b1 on the host (identity when gamma=1,
beta=0). Padded capacity slots carry weight 0 and gather token 0, so they
contribute exact zeros.
"""
import sys

sys.path.insert(0, "/opt/trn_rl_repo")

import numpy as np

B, S, H = 4, 2048, 1024
E, K_TOP, F = 8, 2, 4096
LN_EPS = 1e-5
N_CORES = 8
P = 128
T_TOTAL = B * S                      # 8192 tokens
CAP = 2304                           # per-expert capacity (mean 2048, +6 sigma)
CHUNKS = [512, 512, 512, 512, 256]   # token chunks per device pass
assert sum(CHUNKS) == CAP
HT = H // P                          # 8
FT = F // P                          # 32
W2_FB = 8                            # w2 streamed in blocks of 8 f-tiles

_runtime_cache = {}


def build_nc(bench_iters=1):
    """Build + compile the SPMD device program (same NEFF for all 8 cores).

    bench_iters > 1 wraps the body in a hardware For_i loop for timing.
    """
    import concourse.bass as bass
    import concourse.mybir as mybir
    import concourse.tile as tile
    from concourse import bacc
    from concourse.masks import make_identity

    f32 = mybir.dt.float32
    i32 = mybir.dt.int32
    AF = mybir.ActivationFunctionType

    nc = bacc.Bacc(
        "TRN2", target_bir_lowering=False, debug=False, num_devices=N_CORES
    )
    x_d = nc.declare_dram_parameter("x", [T_TOTAL, H], f32, isOutput=False)
    idx_d = nc.declare_dram_parameter("idx", [CAP], i32, isOutput=False)
    wgt_d = nc.declare_dram_parameter("wgt", [CAP], f32, isOutput=False)
    w1_d = nc.declare_dram_parameter("w1", [H, F], f32, isOutput=False)
    b1_d = nc.declare_dram_parameter("b1", [F], f32, isOutput=False)
    w2_d = nc.declare_dram_parameter("w2", [F, H], f32, isOutput=False)
    y_d = nc.declare_dram_parameter("y", [CAP, H], f32, isOutput=True)

    with tile.TileContext(nc) as tc:
        with (
            tc.tile_pool(name="const", bufs=1) as const_pool,
            tc.tile_pool(name="xg", bufs=1) as xg_pool,
            tc.tile_pool(name="xhat", bufs=1) as xhat_pool,
            tc.tile_pool(name="xhatT", bufs=1) as xhatT_pool,
            tc.tile_pool(name="stats", bufs=4) as stats_pool,
            tc.tile_pool(name="w1s", bufs=3) as w1_pool,
            tc.tile_pool(name="w2s", bufs=3) as w2_pool,
            tc.tile_pool(name="ht", bufs=1) as ht_pool,
            tc.tile_pool(name="yT", bufs=3) as yT_pool,
            tc.tile_pool(name="ytok", bufs=2) as ytok_pool,
            tc.tile_pool(name="wb", bufs=2) as wb_pool,
            tc.tile_pool(name="ps_mm", bufs=3, space="PSUM") as ps_mm,
            tc.tile_pool(name="ps_tp", bufs=4, space="PSUM") as ps_tp,
        ):
            ident = const_pool.tile([P, P], f32)
            make_identity(nc, ident[:])
            eps_t = const_pool.tile([P, 1], f32)
            nc.vector.memset(eps_t[:], LN_EPS)
            idx_sb = const_pool.tile([P, CAP // P], i32)
            nc.sync.dma_start(
                out=idx_sb[:], in_=idx_d.ap().rearrange("(g p) -> p g", p=P)
            )
            b1_sb = const_pool.tile([P, FT], f32)
            nc.sync.dma_start(
                out=b1_sb[:], in_=b1_d.ap().rearrange("(t p) -> p t", p=P)
            )

            x_ap = x_d.ap()
            w1_ap = w1_d.ap().rearrange("(ho p) f -> p ho f", p=P)  # [128,8,4096]
            w2_ap = w2_d.ap().rearrange("(fo p) h -> p fo h", p=P)  # [128,32,1024]
            wgt_ap = wgt_d.ap()
            y_full = y_d.ap()

            def body(_iv=None):
                off = 0
                for tc_sz in CHUNKS:
                    TG = tc_sz // P
                    g0 = off // P
                    # ---- gather this chunk's tokens (token-major) ----
                    xg = xg_pool.tile([P, TG, H], f32, tag="xg")
                    for tg in range(TG):
                        nc.gpsimd.indirect_dma_start(
                            out=xg[:, tg, :],
                            out_offset=None,
                            in_=x_ap,
                            in_offset=bass.IndirectOffsetOnAxis(
                                ap=idx_sb[:, g0 + tg : g0 + tg + 1], axis=0
                            ),
                        )
                    # ---- layernorm ----
                    xhat = xhat_pool.tile([P, TG, H], f32, tag="xhat")
                    for tg in range(TG):
                        st = stats_pool.tile([P, 2, 6], f32, tag="st")
                        nc.vector.bn_stats(out=st[:, 0, :], in_=xg[:, tg, 0:512])
                        nc.vector.bn_stats(out=st[:, 1, :], in_=xg[:, tg, 512:1024])
                        mv = stats_pool.tile([P, 2], f32, tag="mv")
                        nc.vector.bn_aggr(out=mv[:], in_=st[:])
                        rstd = stats_pool.tile([P, 1], f32, tag="rstd")
                        nc.scalar.activation(
                            out=rstd[:], in_=mv[:, 1:2], func=AF.Sqrt,
                            bias=eps_t[:], scale=1.0,
                        )
                        nc.vector.reciprocal(out=rstd[:], in_=rstd[:])
                        nmr = stats_pool.tile([P, 1], f32, tag="nmr")
                        nc.vector.tensor_mul(out=nmr[:], in0=mv[:, 0:1], in1=rstd[:])
                        nc.vector.tensor_scalar_mul(nmr[:], nmr[:], -1.0)
                        nc.scalar.activation(
                            out=xhat[:, tg, :], in_=xg[:, tg, :], func=AF.Identity,
                            bias=nmr[:], scale=rstd[:],
                        )
                    # ---- transpose to feature-major [h, t] ----
                    xhatT = xhatT_pool.tile([P, HT, tc_sz], f32, tag="xhatT")
                    for tg in range(TG):
                        for ho in range(HT):
                            tp = ps_tp.tile([P, P], f32, tag="tp")
                            nc.tensor.transpose(
                                tp[:], xhat[:, tg, ho * P : (ho + 1) * P], ident[:]
                            )
                            nc.vector.tensor_copy(
                                out=xhatT[:, ho, tg * P : (tg + 1) * P], in_=tp[:]
                            )
                    # ---- per-token combine weights, broadcast on partitions ----
                    wb = wb_pool.tile([P, tc_sz], f32, tag="wb")
                    nc.sync.dma_start(
                        out=wb[:],
                        in_=wgt_ap[off : off + tc_sz][None, :].to_broadcast(
                            [P, tc_sz]
                        ),
                    )
                    # ---- h = gelu(xn @ w1 + b1), feature-major ----
                    htile = ht_pool.tile([P, FT, tc_sz], f32, tag="ht")
                    for ft in range(FT):
                        w1sb = w1_pool.tile([P, HT, P], f32, tag="w1")
                        nc.sync.dma_start(
                            out=w1sb[:], in_=w1_ap[:, :, ft * P : (ft + 1) * P]
                        )
                        pm = ps_mm.tile([P, tc_sz], f32, tag="mm")
                        for ho in range(HT):
                            nc.tensor.matmul(
                                pm[:],
                                lhsT=w1sb[:, ho, :],
                                rhs=xhatT[:, ho, :],
                                start=(ho == 0),
                                stop=(ho == HT - 1),
                            )
                        nc.scalar.activation(
                            out=htile[:, ft, :], in_=pm[:], func=AF.Gelu_apprx_tanh,
                            bias=b1_sb[:, ft : ft + 1], scale=1.0,
                        )
                    # ---- y^T = wgt * (h @ w2), then transpose to token-major ----
                    ytok = ytok_pool.tile([P, TG, H], f32, tag="ytok")
                    for ho in range(HT):
                        pm2 = ps_mm.tile([P, tc_sz], f32, tag="mm")
                        for fb in range(FT // W2_FB):
                            w2sb = w2_pool.tile([P, W2_FB, P], f32, tag="w2")
                            nc.sync.dma_start(
                                out=w2sb[:],
                                in_=w2_ap[
                                    :,
                                    fb * W2_FB : (fb + 1) * W2_FB,
                                    ho * P : (ho + 1) * P,
                                ],
                            )
                            for fi in range(W2_FB):
                                fo = fb * W2_FB + fi
                                nc.tensor.matmul(
                                    pm2[:],
                                    lhsT=w2sb[:, fi, :],
                                    rhs=htile[:, fo, :],
                                    start=(fo == 0),
                                    stop=(fo == FT - 1),
                                )
                        yT = yT_pool.tile([P, tc_sz], f32, tag="yT")
                        nc.vector.tensor_mul(out=yT[:], in0=pm2[:], in1=wb[:])
                        for tg in range(TG):
                            tp2 = ps_tp.tile([P, P], f32, tag="tp")
                            nc.tensor.transpose(
                                tp2[:], yT[:, tg * P : (tg + 1) * P], ident[:]
                            )
                            nc.vector.tensor_copy(
                                out=ytok[:, tg, ho * P : (ho + 1) * P], in_=tp2[:]
                            )
                    y_view = y_full[off : off + tc_sz].rearrange(
                        "(g p) h -> p g h", p=P
                    )
                    nc.sync.dma_start(out=y_view, in_=ytok[:])
                    off += tc_sz

            if bench_iters == 1:
                body()
            else:
                with tc.For_i(0, bench_iters, 1) as iv:
                    body(iv)

    nc.compile()
    return nc


def get_nc(bench_iters=1):
    key = bench_iters
    if key not in _runtime_cache:
        _runtime_cache[key] = build_nc(bench_iters)
    return _runtime_cache[key]


def host_router(x2d, router_w, router_b):
    """Replicates the reference router: softmax -> top-2 -> renormalize.

    Returns per-expert padded index/weight arrays plus overflow pairs
    (token, expert, weight) that exceeded CAP (vanishingly rare).
    """
    logits = x2d @ router_w + router_b                      # [T, E]
    top2 = np.argpartition(-logits, 1, axis=1)[:, :2]       # [T, 2]
    l2 = np.take_along_axis(logits, top2, axis=1)
    e2 = np.exp(l2 - l2.max(1, keepdims=True))
    w2 = (e2 / e2.sum(1, keepdims=True)).astype(np.float32)  # renormalized

    idx_all = np.zeros((E, CAP), np.int32)
    wgt_all = np.zeros((E, CAP), np.float32)
    overflow = []
    for e in range(E):
        tok, slot = np.nonzero(top2 == e)
        wts = w2[tok, slot]
        n = len(tok)
        if n > CAP:
            for t, w in zip(tok[CAP:], wts[CAP:]):
                overflow.append((int(t), e, float(w)))
            tok, wts, n = tok[:CAP], wts[:CAP], CAP
        idx_all[e, :n] = tok
        wgt_all[e, :n] = wts
    return idx_all, wgt_all, overflow


def _expert_ffn_host(x_rows, g, b, w1, b1, w2, b2):
    """Reference expert FFN on host (numpy) for rare overflow tokens."""
    mu = x_rows.mean(-1, keepdims=True)
    var = ((x_rows - mu) ** 2).mean(-1, keepdims=True)
    xn = (x_rows - mu) / np.sqrt(var + LN_EPS) * g + b
    t = xn @ w1 + b1
    h = 0.5 * t * (1.0 + np.tanh(np.sqrt(2.0 / np.pi) * (t + 0.044715 * t**3)))
    return h @ w2 + b2


def kernel(hidden_states, router_w, router_b, ln_g, ln_b, w1, b1, w2, b2):
    from concourse.bass_utils import run_bass_kernel_spmd

    x2d = np.ascontiguousarray(
        np.asarray(hidden_states, np.float32).reshape(T_TOTAL, H)
    )
    router_w = np.asarray(router_w, np.float32)
    router_b = np.asarray(router_b, np.float32)
    ln_g = np.asarray(ln_g, np.float32)
    ln_b = np.asarray(ln_b, np.float32)
    w1 = np.asarray(w1, np.float32)
    b1 = np.asarray(b1, np.float32)
    w2 = np.asarray(w2, np.float32)
    b2 = np.asarray(b2, np.float32)

    idx_all, wgt_all, overflow = host_router(x2d, router_w, router_b)

    # Fold LN affine params into w1/b1 (identity for the spec's ones/zeros fill)
    need_fold = not (np.all(ln_g == 1.0) and np.all(ln_b == 0.0))
    in_maps = []
    for e in range(E):
        w1_e = ln_g[e][:, None] * w1[e] if need_fold else w1[e]
        b1_e = b1[e] + ln_b[e] @ w1[e] if need_fold else b1[e]
        in_maps.append(
            dict(
                x=x2d,
                idx=idx_all[e],
                wgt=wgt_all[e],
                w1=np.ascontiguousarray(w1_e),
                b1=np.ascontiguousarray(b1_e),
                w2=np.ascontiguousarray(w2[e]),
            )
        )

    nc = get_nc(1)
    res = run_bass_kernel_spmd(nc, in_maps, list(range(N_CORES))).results

    # host combine: out = x + sum_e scatter(y_e) (+ wgt * b2 if b2 != 0)
    out_ext = np.zeros((T_TOTAL + 1, H), np.float32)  # row T_TOTAL = pad sink
    out_ext[:T_TOTAL] = x2d
    has_b2 = bool(np.any(b2))
    for e in range(E):
        y_e = res[e]["y"]
        if has_b2:
            y_e = y_e + wgt_all[e][:, None] * b2[e][None, :]
        scatter_idx = np.where(wgt_all[e] > 0, idx_all[e], T_TOTAL)
        out_ext[scatter_idx] += y_e
    for t, e, w in overflow:
        # _expert_ffn_host returns the residual-free tail h@w2 + b2; the
        # residual x is added once globally (top-2 weights sum to 1).
        tail = _expert_ffn_host(
            x2d[t : t + 1], ln_g[e], ln_b[e], w1[e], b1[e], w2[e], b2[e]
        )
        out_ext[t] += np.float32(w) * tail[0]
    combined = out_ext[:T_TOTAL].reshape(B, S, H)
    zero = np.zeros((), np.float32)
    return combined, zero.copy(), zero.copy()


# revision 5
# speedup vs baseline: 3.4121x; 3.4121x over previous
"""MoE ACT block (nn_MoEACTBlock_62225486185201) on 8 Trainium2 NeuronCores.

Strategy: expert-parallel with top-2 routing sparsity.
  - Host: router matmul + top-2 + renormalize (0.01% of FLOPs), builds
    per-expert token index lists padded to a fixed capacity.
  - Device (core e = expert e): indirect-DMA gather of its tokens, LayerNorm,
    PE transpose to feature-major, h = gelu(xn @ w1 + b1), y = w_tok * (h @ w2),
    PE transpose back to token-major, write y rows.
  - Host: combine out = x + sum_e scatter(y_e) (+ b2 term), since each token is
    processed by exactly 2 experts living on different cores.

LayerNorm gamma/beta are folded into w1/b1 on the host (identity when gamma=1,
beta=0). Padded capacity slots carry weight 0 and gather token 0, so they
contribute exact zeros.
"""
import sys

sys.path.insert(0, "/opt/trn_rl_repo")

import numpy as np

B, S, H = 4, 2048, 1024
E, K_TOP, F = 8, 2, 4096
LN_EPS = 1e-5
N_CORES = 8
P = 128
T_TOTAL = B * S                      # 8192 tokens
CAP = 2304                           # per-expert capacity (mean 2048, +6 sigma)
CHUNKS = [512, 512, 512, 512, 256]   # token chunks per device pass
assert sum(CHUNKS) == CAP
HT = H // P                          # 8
FT = F // P                          # 32
W2_FB = 8                            # w2 streamed in blocks of 8 f-tiles

_runtime_cache = {}


def build_nc(bench_iters=1, mm1_r=True, mm2_r=True):
    """Build + compile the SPMD device program (same NEFF for all 8 cores).

    bench_iters > 1 wraps the body in a hardware For_i loop for timing.
    """
    import concourse.bass as bass
    import concourse.mybir as mybir
    import concourse.tile as tile
    from concourse import bacc
    from concourse.masks import make_identity

    f32 = mybir.dt.float32
    f32r = mybir.dt.float32r
    i32 = mybir.dt.int32
    AF = mybir.ActivationFunctionType

    mm1_dt = f32r if mm1_r else f32
    mm2_dt = f32r if mm2_r else f32
    nc = bacc.Bacc(
        "TRN2", target_bir_lowering=False, debug=False, num_devices=N_CORES
    )
    x_d = nc.declare_dram_parameter("x", [T_TOTAL, H], f32, isOutput=False)
    idx_d = nc.declare_dram_parameter("idx", [CAP], i32, isOutput=False)
    wgt_d = nc.declare_dram_parameter("wgt", [CAP], f32, isOutput=False)
    w1_d = nc.declare_dram_parameter("w1", [H, F], mm1_dt, isOutput=False)
    b1_d = nc.declare_dram_parameter("b1", [F], f32, isOutput=False)
    w2_d = nc.declare_dram_parameter("w2", [F, H], mm2_dt, isOutput=False)
    y_d = nc.declare_dram_parameter("y", [CAP, H], f32, isOutput=True)

    with tile.TileContext(nc) as tc:
        with (
            tc.tile_pool(name="const", bufs=1) as const_pool,
            tc.tile_pool(name="xg", bufs=1) as xg_pool,
            tc.tile_pool(name="xhat", bufs=1) as xhat_pool,
            tc.tile_pool(name="xhatT", bufs=1) as xhatT_pool,
            tc.tile_pool(name="stats", bufs=4) as stats_pool,
            tc.tile_pool(name="w1s", bufs=3) as w1_pool,
            tc.tile_pool(name="w2s", bufs=3) as w2_pool,
            tc.tile_pool(name="ht", bufs=1) as ht_pool,
            tc.tile_pool(name="yT", bufs=3) as yT_pool,
            tc.tile_pool(name="ytok", bufs=2) as ytok_pool,
            tc.tile_pool(name="wb", bufs=2) as wb_pool,
            tc.tile_pool(name="ps_mm", bufs=3, space="PSUM") as ps_mm,
            tc.tile_pool(name="ps_tp", bufs=4, space="PSUM") as ps_tp,
        ):
            ident = const_pool.tile([P, P], f32)
            make_identity(nc, ident[:])
            eps_t = const_pool.tile([P, 1], f32)
            nc.vector.memset(eps_t[:], LN_EPS)
            idx_sb = const_pool.tile([P, CAP // P], i32)
            nc.sync.dma_start(
                out=idx_sb[:], in_=idx_d.ap().rearrange("(g p) -> p g", p=P)
            )
            b1_sb = const_pool.tile([P, FT], f32)
            nc.sync.dma_start(
                out=b1_sb[:], in_=b1_d.ap().rearrange("(t p) -> p t", p=P)
            )

            x_ap = x_d.ap()
            w1_ap = w1_d.ap().rearrange("(ho p) f -> p ho f", p=P)  # [128,8,4096]
            w2_ap = w2_d.ap().rearrange("(fo p) h -> p fo h", p=P)  # [128,32,1024]
            wgt_ap = wgt_d.ap()
            y_full = y_d.ap()

            def body(_iv=None):
                off = 0
                for tc_sz in CHUNKS:
                    TG = tc_sz // P
                    g0 = off // P
                    # ---- gather this chunk's tokens (token-major) ----
                    xg = xg_pool.tile([P, TG, H], f32, tag="xg")
                    for tg in range(TG):
                        nc.gpsimd.indirect_dma_start(
                            out=xg[:, tg, :],
                            out_offset=None,
                            in_=x_ap,
                            in_offset=bass.IndirectOffsetOnAxis(
                                ap=idx_sb[:, g0 + tg : g0 + tg + 1], axis=0
                            ),
                        )
                    # ---- layernorm ----
                    xhat = xhat_pool.tile([P, TG, H], f32, tag="xhat")
                    for tg in range(TG):
                        st = stats_pool.tile([P, 2, 6], f32, tag="st")
                        nc.vector.bn_stats(out=st[:, 0, :], in_=xg[:, tg, 0:512])
                        nc.vector.bn_stats(out=st[:, 1, :], in_=xg[:, tg, 512:1024])
                        mv = stats_pool.tile([P, 2], f32, tag="mv")
                        nc.vector.bn_aggr(out=mv[:], in_=st[:])
                        rstd = stats_pool.tile([P, 1], f32, tag="rstd")
                        nc.scalar.activation(
                            out=rstd[:], in_=mv[:, 1:2], func=AF.Sqrt,
                            bias=eps_t[:], scale=1.0,
                        )
                        nc.vector.reciprocal(out=rstd[:], in_=rstd[:])
                        nmr = stats_pool.tile([P, 1], f32, tag="nmr")
                        nc.vector.tensor_mul(out=nmr[:], in0=mv[:, 0:1], in1=rstd[:])
                        nc.vector.tensor_scalar_mul(nmr[:], nmr[:], -1.0)
                        nc.scalar.activation(
                            out=xhat[:, tg, :], in_=xg[:, tg, :], func=AF.Identity,
                            bias=nmr[:], scale=rstd[:],
                        )
                    # ---- transpose to feature-major [h, t] ----
                    xhatT = xhatT_pool.tile([P, HT, tc_sz], mm1_dt, tag="xhatT")
                    for tg in range(TG):
                        for ho in range(HT):
                            tp = ps_tp.tile([P, P], f32, tag="tp")
                            nc.tensor.transpose(
                                tp[:], xhat[:, tg, ho * P : (ho + 1) * P], ident[:]
                            )
                            nc.vector.tensor_copy(
                                out=xhatT[:, ho, tg * P : (tg + 1) * P], in_=tp[:]
                            )
                    # ---- per-token combine weights, broadcast on partitions ----
                    wb = wb_pool.tile([P, tc_sz], f32, tag="wb")
                    nc.sync.dma_start(
                        out=wb[:],
                        in_=wgt_ap[off : off + tc_sz][None, :].to_broadcast(
                            [P, tc_sz]
                        ),
                    )
                    # ---- h = gelu(xn @ w1 + b1), feature-major ----
                    htile = ht_pool.tile([P, FT, tc_sz], mm2_dt, tag="ht")
                    for ft in range(FT):
                        w1sb = w1_pool.tile([P, HT, P], mm1_dt, tag="w1")
                        nc.sync.dma_start(
                            out=w1sb[:], in_=w1_ap[:, :, ft * P : (ft + 1) * P]
                        )
                        pm = ps_mm.tile([P, tc_sz], f32, tag="mm")
                        for ho in range(HT):
                            nc.tensor.matmul(
                                pm[:],
                                lhsT=w1sb[:, ho, :],
                                rhs=xhatT[:, ho, :],
                                start=(ho == 0),
                                stop=(ho == HT - 1),
                            )
                        nc.scalar.activation(
                            out=htile[:, ft, :], in_=pm[:], func=AF.Gelu_apprx_tanh,
                            bias=b1_sb[:, ft : ft + 1], scale=1.0,
                        )
                    # ---- y^T = wgt * (h @ w2), then transpose to token-major ----
                    ytok = ytok_pool.tile([P, TG, H], f32, tag="ytok")
                    for ho in range(HT):
                        pm2 = ps_mm.tile([P, tc_sz], f32, tag="mm")
                        for fb in range(FT // W2_FB):
                            w2sb = w2_pool.tile([P, W2_FB, P], mm2_dt, tag="w2")
                            nc.sync.dma_start(
                                out=w2sb[:],
                                in_=w2_ap[
                                    :,
                                    fb * W2_FB : (fb + 1) * W2_FB,
                                    ho * P : (ho + 1) * P,
                                ],
                            )
                            for fi in range(W2_FB):
                                fo = fb * W2_FB + fi
                                nc.tensor.matmul(
                                    pm2[:],
                                    lhsT=w2sb[:, fi, :],
                                    rhs=htile[:, fo, :],
                                    start=(fo == 0),
                                    stop=(fo == FT - 1),
                                )
                        yT = yT_pool.tile([P, tc_sz], f32, tag="yT")
                        nc.vector.tensor_mul(out=yT[:], in0=pm2[:], in1=wb[:])
                        for tg in range(TG):
                            tp2 = ps_tp.tile([P, P], f32, tag="tp")
                            nc.tensor.transpose(
                                tp2[:], yT[:, tg * P : (tg + 1) * P], ident[:]
                            )
                            nc.vector.tensor_copy(
                                out=ytok[:, tg, ho * P : (ho + 1) * P], in_=tp2[:]
                            )
                    y_view = y_full[off : off + tc_sz].rearrange(
                        "(g p) h -> p g h", p=P
                    )
                    nc.sync.dma_start(out=y_view, in_=ytok[:])
                    off += tc_sz

            if bench_iters == 1:
                body()
            else:
                with tc.For_i(0, bench_iters, 1) as iv:
                    body(iv)

    nc.compile()
    return nc


def get_nc(bench_iters=1, **kw):
    key = (bench_iters, tuple(sorted(kw.items())))
    if key not in _runtime_cache:
        _runtime_cache[key] = build_nc(bench_iters, **kw)
    return _runtime_cache[key]


def host_router(x2d, router_w, router_b):
    """Replicates the reference router: softmax -> top-2 -> renormalize.

    Returns per-expert padded index/weight arrays plus overflow pairs
    (token, expert, weight) that exceeded CAP (vanishingly rare).
    """
    logits = x2d @ router_w + router_b                      # [T, E]
    top2 = np.argpartition(-logits, 1, axis=1)[:, :2]       # [T, 2]
    l2 = np.take_along_axis(logits, top2, axis=1)
    e2 = np.exp(l2 - l2.max(1, keepdims=True))
    w2 = (e2 / e2.sum(1, keepdims=True)).astype(np.float32)  # renormalized

    idx_all = np.zeros((E, CAP), np.int32)
    wgt_all = np.zeros((E, CAP), np.float32)
    overflow = []
    for e in range(E):
        tok, slot = np.nonzero(top2 == e)
        wts = w2[tok, slot]
        n = len(tok)
        if n > CAP:
            for t, w in zip(tok[CAP:], wts[CAP:]):
                overflow.append((int(t), e, float(w)))
            tok, wts, n = tok[:CAP], wts[:CAP], CAP
        idx_all[e, :n] = tok
        wgt_all[e, :n] = wts
    return idx_all, wgt_all, overflow


def _expert_ffn_host(x_rows, g, b, w1, b1, w2, b2):
    """Reference expert FFN on host (numpy) for rare overflow tokens."""
    mu = x_rows.mean(-1, keepdims=True)
    var = ((x_rows - mu) ** 2).mean(-1, keepdims=True)
    xn = (x_rows - mu) / np.sqrt(var + LN_EPS) * g + b
    t = xn @ w1 + b1
    h = 0.5 * t * (1.0 + np.tanh(np.sqrt(2.0 / np.pi) * (t + 0.044715 * t**3)))
    return h @ w2 + b2


def kernel(hidden_states, router_w, router_b, ln_g, ln_b, w1, b1, w2, b2):
    from concourse.bass_utils import run_bass_kernel_spmd

    x2d = np.ascontiguousarray(
        np.asarray(hidden_states, np.float32).reshape(T_TOTAL, H)
    )
    router_w = np.asarray(router_w, np.float32)
    router_b = np.asarray(router_b, np.float32)
    ln_g = np.asarray(ln_g, np.float32)
    ln_b = np.asarray(ln_b, np.float32)
    w1 = np.asarray(w1, np.float32)
    b1 = np.asarray(b1, np.float32)
    w2 = np.asarray(w2, np.float32)
    b2 = np.asarray(b2, np.float32)

    idx_all, wgt_all, overflow = host_router(x2d, router_w, router_b)

    # Fold LN affine params into w1/b1 (identity for the spec's ones/zeros fill)
    need_fold = not (np.all(ln_g == 1.0) and np.all(ln_b == 0.0))
    in_maps = []
    for e in range(E):
        w1_e = ln_g[e][:, None] * w1[e] if need_fold else w1[e]
        b1_e = b1[e] + ln_b[e] @ w1[e] if need_fold else b1[e]
        in_maps.append(
            dict(
                x=x2d,
                idx=idx_all[e],
                wgt=wgt_all[e],
                w1=np.ascontiguousarray(w1_e),
                b1=np.ascontiguousarray(b1_e),
                w2=np.ascontiguousarray(w2[e]),
            )
        )

    nc = get_nc(1)
    res = run_bass_kernel_spmd(nc, in_maps, list(range(N_CORES))).results

    # host combine: out = x + sum_e scatter(y_e) (+ wgt * b2 if b2 != 0)
    out_ext = np.zeros((T_TOTAL + 1, H), np.float32)  # row T_TOTAL = pad sink
    out_ext[:T_TOTAL] = x2d
    has_b2 = bool(np.any(b2))
    for e in range(E):
        y_e = res[e]["y"]
        if has_b2:
            y_e = y_e + wgt_all[e][:, None] * b2[e][None, :]
        scatter_idx = np.where(wgt_all[e] > 0, idx_all[e], T_TOTAL)
        out_ext[scatter_idx] += y_e
    for t, e, w in overflow:
        # _expert_ffn_host returns the residual-free tail h@w2 + b2; the
        # residual x is added once globally (top-2 weights sum to 1).
        tail = _expert_ffn_host(
            x2d[t : t + 1], ln_g[e], ln_b[e], w1[e], b1[e], w2[e], b2[e]
        )
        out_ext[t] += np.float32(w) * tail[0]
    combined = out_ext[:T_TOTAL].reshape(B, S, H)
    zero = np.zeros((), np.float32)
    return combined, zero.copy(), zero.copy()
